# revision 4
# baseline (speedup 1.0000x reference)
"""FakeQuantLinear Trainium2 kernel (8-core data-parallel over tokens).

Math (per reference):
    x_int8 = clip(round(x / scale_a) + zp_a, -128, 127)
    y = (x_int8 - zp_a) @ (W - w_zp).T * (scale_a * w_scale) + bias

Key facts exploited:
  * fp16 has an 11-bit significand: adding OFF = 1536 + zp_a (1536 =
    1.5*2^10, the fp16 round-to-nearest-even shifter) to x/scale_a and
    casting f32->fp16 rounds to an exact integer + OFF for |v| <= 384.
    clip(round(v)+zp, -128, 127) + 1536 == clip(round(v)+OFF, 1408, 1663)
    (bounds independent of zp). The +OFF offset is carried THROUGH the
    matmul and folded out via host-precomputed weight row-sums:
        y = (xq - OFF) @ Wc.T * ts + b  ==  xq @ Wc.T * ts + (b - ts*OFF*rowsum(Wc))
    so quantization is 2 elementwise passes (scale+shift, clamp) instead
    of 3 (scale+shift, clamp-hi, clamp-lo+unshift).
  * fp16 matmul runs at bf16 speed (1 elem/cell/cycle); small-int
    operands make products exact; f32 PSUM accumulation error is ~1e-4
    relative even with the ~1536-offset operands.

Sharding: data-parallel over the 8192 tokens; each of the 8 cores handles
1024 tokens and holds the full (host-pre-centered, fp16) weight.

Schedule (the perf-critical part):
  * Phase 1 streams x k-tiles in; pass1 (t = fp16(x*inv_s + OFF))
    alternates ACT/DVE per tile, pass2 (clamp, fp16->fp16) stays on DVE.
  * Group 0 (G=4 o-tiles x 2 psum halves = all 8 PSUM banks) runs
    k-interleaved, consuming xq tiles as they arrive -> the PE has
    ~1.7us of matmul work per quantized tile vs ~1.2us quant cost, so
    the PE, not the quantizer, is the critical path almost immediately.
  * Groups 1..7 run as per-(o-tile, half) streams: each PSUM bank's full
    k-loop completes and drains while the next bank's matmuls run, so
    drains/stores never gap the PE.
  * Weight DMA rides the otherwise-idle GPSIMD queue; x/y/bias ride SYNC.
"""

import sys

for _p in ("/opt/trn_rl_repo",):
    if _p not in sys.path:
        sys.path.insert(0, _p)

import numpy as np
import ml_dtypes

B, S, IN, OUT = 4, 2048, 4096, 4096
M = B * S  # 8192 tokens
NCORES = 8
MS = M // NCORES  # 1024 tokens per core
FP16_MAGIC = 1536.0  # 1.5 * 2**10: fp16 round-to-nearest-even shifter
CLAMP_LO = FP16_MAGIC - 128.0  # 1408
CLAMP_HI = FP16_MAGIC + 127.0  # 1663


def build_bass(ms, in_dim, out_dim, inv_s, off, total_scale,
               G=4, w_prefetch=8, fast_start=2, loop_n=1, mh=None,
               act_pass1=2, w_dma="gpsimd", w_bufs=10):
    """Build the per-core Bass/Tile program.

    ms: tokens on this core; in_dim/out_dim: contraction / output features.
    inv_s, off, total_scale: compile-time immediates (off = 1536 + zp_a).
    act_pass1: every act_pass1-th tile's pass1 runs on ACT (0 = all DVE).
    mh: psum tile width (None -> ms // 2).
    """
    import concourse.bass as bass
    import concourse.mybir as mybir
    import concourse.tile as tile
    from concourse import bacc

    kt = in_dim // 128  # k tiles
    ot = out_dim // 128  # o tiles
    if mh is None:
        mh = ms // 2
    nh = ms // mh  # psum halves per o-tile
    assert ot % G == 0 and G * nh <= 8

    f32 = mybir.dt.float32
    f16 = mybir.dt.float16
    Act = mybir.ActivationFunctionType
    Alu = mybir.AluOpType

    nc = bacc.Bacc()
    xT_d = nc.dram_tensor("xT", [in_dim, ms], f32, kind="ExternalInput")
    wp_d = nc.dram_tensor("wp", [ot, 128, in_dim], f16, kind="ExternalInput")
    bias_d = nc.dram_tensor("biasc", [128, ot], f32, kind="ExternalInput")
    yT_d = nc.dram_tensor("yT", [out_dim, ms], f32, kind="ExternalOutput")

    with tile.TileContext(nc) as tc:
        with (
            tc.tile_pool(name="xin", bufs=3) as xin_pool,
            tc.tile_pool(name="tmp", bufs=3) as tmp_pool,
            tc.tile_pool(name="xq", bufs=1) as xq_pool,
            tc.tile_pool(name="wts", bufs=w_bufs) as w_pool,
            tc.tile_pool(name="psum", bufs=G * nh, space="PSUM") as psum_pool,
            tc.tile_pool(name="yout", bufs=3) as y_pool,
            tc.tile_pool(name="const", bufs=1) as const_pool,
        ):
            w_eng = getattr(nc, w_dma)

            def emit_body():
                bias_sb = const_pool.tile([128, ot], f32, tag="bias", name="bias_sb")
                nc.sync.dma_start(out=bias_sb[:], in_=bias_d[:])
                off_sb = const_pool.tile([128, 1], f32, tag="off", name="off_sb")
                nc.vector.memset(off_sb[:], float(off))

                # First w blocks interleave with the leading x tiles so the
                # PE's first LDWs aren't queued behind the whole x stream.
                wb_pre = {}

                def prefetch_w(j, chunks=1):
                    wb = w_pool.tile([128, in_dim], f16, tag="wb", name=f"wbp_{j}")
                    cw = in_dim // chunks
                    for c in range(chunks):
                        w_eng.dma_start(
                            out=wb[:, c * cw : (c + 1) * cw],
                            in_=wp_d[j][:, c * cw : (c + 1) * cw],
                        )
                    wb_pre[j] = wb

                if w_prefetch > 0:
                    prefetch_w(0, chunks=4 if fast_start else 1)

                # Phase 1: quantize x -> fp16 (integer + OFF), SBUF-resident.
                xq_tiles = []
                for k in range(kt):
                    if k % 2 == 1 and 1 + k // 2 < w_prefetch:
                        prefetch_w(1 + k // 2)
                    xf = xin_pool.tile([128, ms], f32)
                    t1 = tmp_pool.tile([128, ms], f16, tag="t1")
                    xq = xq_pool.tile([128, ms], f16, tag=f"xq{k}")
                    # First tiles are processed in m-halves so the first
                    # matmul's rhs is ready sooner (subtile deps).
                    splits = (
                        [(0, ms // 2), (ms // 2, ms)] if k < fast_start else [(0, ms)]
                    )
                    for c0, c1 in splits:
                        s = slice(c0, c1)
                        nc.sync.dma_start(
                            out=xf[:, s], in_=xT_d[k * 128 : (k + 1) * 128, s]
                        )
                        # pass1: t1 = fp16(x * (1/s) + OFF)  (f32 math, the
                        # fp16 output cast does the round-to-integer)
                        if act_pass1 and k % act_pass1 == 0:
                            nc.scalar.activation(
                                t1[:, s], xf[:, s], Act.Identity,
                                bias=off_sb[:, 0:1], scale=inv_s,
                            )
                        else:
                            nc.vector.tensor_scalar(
                                out=t1[:, s], in0=xf[:, s], scalar1=inv_s,
                                scalar2=float(off), op0=Alu.mult, op1=Alu.add,
                            )
                        # pass2: xq = clamp(t1, 1408, 1663)  (fp16->fp16)
                        nc.vector.tensor_scalar(
                            out=xq[:, s], in0=t1[:, s], scalar1=CLAMP_HI,
                            scalar2=CLAMP_LO, op0=Alu.min, op1=Alu.max,
                        )
                    xq_tiles.append(xq)

                # Phase 2: matmuls. yT[o128, ms] = wb.T @ xq per o-tile.
                for jg in range(ot // G):
                    js = [jg * G + i for i in range(G)]
                    wbs = []
                    for j in js:
                        if j in wb_pre:
                            wb = wb_pre.pop(j)
                        else:
                            wb = w_pool.tile(
                                [128, in_dim], f16, tag="wb", name=f"wb_{j}"
                            )
                            w_eng.dma_start(out=wb[:], in_=wp_d[j])
                        wbs.append(wb)

                    if jg == 0:
                        # k-interleaved: consume xq tiles as they arrive,
                        # all G*nh PSUM banks accumulate in parallel.
                        pss = []
                        for i, j in enumerate(js):
                            pss.append([
                                psum_pool.tile(
                                    [128, mh], f32, tag="ps", name=f"ps_{j}_{h}"
                                )
                                for h in range(nh)
                            ])
                        for k in range(kt):
                            st, sp = (k == 0), (k == kt - 1)
                            for i in range(G):
                                lhs = wbs[i][:, k * 128 : (k + 1) * 128]
                                for h in range(nh):
                                    nc.tensor.matmul(
                                        pss[i][h][:], lhs,
                                        xq_tiles[k][:, h * mh : (h + 1) * mh],
                                        start=st, stop=sp,
                                    )
                        for i, j in enumerate(js):
                            y = y_pool.tile([128, ms], f32, tag="y", name=f"y_{j}")
                            for h in range(nh):
                                hs = slice(h * mh, (h + 1) * mh)
                                nc.scalar.activation(
                                    y[:, hs], pss[i][h][:], Act.Identity,
                                    bias=bias_sb[:, j : j + 1], scale=total_scale,
                                )
                                nc.sync.dma_start(
                                    out=yT_d[j * 128 : (j + 1) * 128, hs],
                                    in_=y[:, hs],
                                )
                        continue

                    # Streams: each PSUM bank's whole k-loop runs, then its
                    # drain + store overlap the next bank's matmuls.
                    for i, j in enumerate(js):
                        y = y_pool.tile([128, ms], f32, tag="y", name=f"y_{j}")
                        for h in range(nh):
                            hs = slice(h * mh, (h + 1) * mh)
                            ps = psum_pool.tile(
                                [128, mh], f32, tag="ps", name=f"ps_{j}_{h}"
                            )
                            for k in range(kt):
                                nc.tensor.matmul(
                                    ps[:], wbs[i][:, k * 128 : (k + 1) * 128],
                                    xq_tiles[k][:, hs],
                                    start=(k == 0), stop=(k == kt - 1),
                                )
                            nc.scalar.activation(
                                y[:, hs], ps[:], Act.Identity,
                                bias=bias_sb[:, j : j + 1], scale=total_scale,
                            )
                            nc.sync.dma_start(
                                out=yT_d[j * 128 : (j + 1) * 128, hs], in_=y[:, hs]
                            )

            if loop_n > 1:
                with tc.For_i(0, loop_n, 1):
                    emit_body()
            else:
                emit_body()

    nc.compile()
    return nc


def prep_inputs(x, weight_int, bias, scale_a, zp_a, weight_scale, weight_zero_point):
    """Host-side layout prep + immediates. Returns (in_maps, immediates)."""
    s_a = float(np.float64(np.asarray(scale_a)))
    zp = int(np.asarray(zp_a))
    s_w = float(np.float64(np.asarray(weight_scale)))
    w_zp = int(np.asarray(weight_zero_point))

    inv_s = float(np.float32(1.0 / np.float64(s_a)))
    off = FP16_MAGIC + zp
    total_scale = float(np.float32(np.float32(s_a) * np.float32(s_w)))

    m, in_dim = x.reshape(-1, x.shape[-1]).shape
    out_dim = weight_int.shape[0]
    ms = m // NCORES
    ot = out_dim // 128

    X = np.ascontiguousarray(x.reshape(m, in_dim).T.astype(np.float32, copy=False))

    # w_prep[j, p, k*128+c] = Wc[j*128+c, k*128+p] (host-centered fp16; the
    # lhsT slice [128k, 128o] = wb[:, k*128:(k+1)*128]).
    w_cent = weight_int.astype(np.int64) - w_zp
    w_prep = np.ascontiguousarray(
        w_cent.astype(np.float16)
        .reshape(ot, 128, in_dim // 128, 128)
        .transpose(0, 3, 2, 1)
    ).reshape(ot, 128, in_dim)

    # Fold the +OFF operand offset out through the weight row-sums:
    # y = xq @ Wc.T * ts + (bias - ts * OFF * rowsum(Wc))
    rowsum = w_cent.sum(axis=1).astype(np.float64)  # [OUT]
    bias_adj = (
        bias.astype(np.float64) - float(total_scale) * float(off) * rowsum
    ).astype(np.float32)
    bias_col = np.ascontiguousarray(bias_adj.reshape(ot, 128).T)

    in_maps = []
    for c in range(NCORES):
        in_maps.append(
            {
                "xT": np.ascontiguousarray(X[:, c * ms : (c + 1) * ms]),
                "wp": w_prep,
                "biasc": bias_col,
            }
        )
    return in_maps, (ms, in_dim, out_dim, inv_s, off, total_scale)


def assemble_output(results, m, out_dim):
    """Concatenate per-core yT shards [OUT, ms] -> y [B, S, OUT]."""
    ys = [np.asarray(r["yT"]).T for r in results]  # each [ms, OUT]
    Y = np.concatenate(ys, axis=0)
    return np.ascontiguousarray(Y.reshape(B, S, out_dim).astype(np.float32))


def run(inputs, trace=False, **spmd_kwargs):
    """Full pipeline returning (y, BassKernelResults). Used by test harness."""
    from concourse.bass_utils import run_bass_kernel_spmd

    in_maps, imm = prep_inputs(**inputs)
    nc = build_bass(*imm)
    res = run_bass_kernel_spmd(
        nc, in_maps, list(range(NCORES)), trace=trace, **spmd_kwargs
    )
    return assemble_output(res.results, M, OUT), res


def kernel(x, weight_int, bias, scale_a, zp_a, weight_scale, weight_zero_point):
    from concourse.bass_utils import run_bass_kernel_spmd

    in_maps, imm = prep_inputs(
        x, weight_int, bias, scale_a, zp_a, weight_scale, weight_zero_point
    )
    nc = build_bass(*imm)
    res = run_bass_kernel_spmd(nc, in_maps, list(range(NCORES)))
    return assemble_output(res.results, M, OUT)
